# revision 1
# baseline (speedup 1.0000x reference)
"""Trainium2 Bass kernel for nn_CamFusionModule (epipolar max-sampling fusion).

Strategy
--------
Data-parallel over output pixels: the 64x64 heatmap grid is split into 8
row-bands of 8 rows, one per NeuronCore (heatmaps replicated, all 12
(curview, othview) pairs on every core, per the sharding hint's
"pair axis splittable / heatmaps replicated" guidance).

Host (jax-cpu, bit-identical to the reference):
  * camera math -> per-pair epipolar sweep coordinates, normalized,
    rounded and clamped exactly as the reference's grid_sample does ->
    fp16 index rows (one row per sweep position t).
  * heatmaps -> per-(pair, sweep, t-pair) stationary gather tables,
    split into fp16 (hi, lo) parts (hi+lo reconstructs ~21 bits), rows
    parity-interleaved (row k = table entry k//2 of sweep position
    2g + k%2) and block-diagonal over parity in the columns.

Device (per NeuronCore), per (pair, sweep):
  * index rows are replicated across all 128 partitions by a
    log-doubling chain of SBUF->SBUF DMAs (chunks of 8 t-pairs), so the
    idle DMA engines do the broadcast and the PE pstate ramp is not
    disturbed by tiny matmuls.
  * per t-pair, a one-hot mask [128, 512] = (P == k//2) is built either
    on DVE (`is_equal` vs a per-partition iota) or on ACT
    (Square(P - iota) -> Relu(1 - sq)), load-balanced.
  * two N=512 fp16 matmuls (hi, lo accumulating in PSUM) gather
    2 samples/column x 16 channels; outputs of 4 t-pairs are stacked
    into one full-width PSUM bank via 32-aligned col groups.
  * DVE running tensor-max over PSUM banks, then partition-block folds
    (small shift DMAs) collapse t-pair slots/parity/sweep.

Output: [12, 16, 512] fp32 per core, reassembled host-side.
"""

import numpy as np
import ml_dtypes

NVIEW = 4
B, C, H, W = 1, 16, 64, 64
HW = H * W
NPAIR = 12
NCORE = 8
PXS = HW // NCORE          # 512 pixels per core
ROWS = H // NCORE          # 8 image rows per core
NTP = W // 2               # 32 t-pairs per sweep
CHTP = 16                  # t-pairs per replication chunk
NCH = NTP // CHTP          # chunks per (pair, sweep)
BIG = 1.0e9                # sentinel for non-finite coords (-> invalid)
ACT_SHARE = 10             # of 32 t-pairs per (pair, sweep) masked on ScalarE

_PAIRS = [(c, o) for c in range(NVIEW) for o in range(NVIEW) if o != c]


def _line_coords(affine_trans, cam_Intri, cam_R, cam_T, inv_affine_trans):
    """Mirror of the reference's fp32 math through the rounded sample
    indices. Returns iy[p, t, px] (x-sweep row index) and ix[p, t, px]
    (y-sweep col index) as float32 [12, 64, 4096], exactly matching the
    reference's `jnp.round((g + 1) * 0.5 * (dim - 1))` values (jax on CPU
    so rounding matches bit-for-bit)."""
    import jax
    import jax.numpy as jnp
    cpu = jax.devices("cpu")[0]
    ctx = jax.default_device(cpu)
    ctx.__enter__()

    V = NVIEW
    h, w = H, W
    yy, xx = jnp.meshgrid(jnp.arange(h, dtype=jnp.float32),
                          jnp.arange(w, dtype=jnp.float32), indexing='ij')
    onehm = jnp.stack([xx.reshape(-1), yy.reshape(-1), jnp.ones(HW, jnp.float32)], 0)
    K = jnp.asarray(cam_Intri).reshape(B, V, 3, 3)
    R = jnp.asarray(cam_R).reshape(B, V, 3, 3)
    T = jnp.asarray(cam_T).reshape(B, V, 3, 1)
    Aff = jnp.asarray(affine_trans).reshape(B, V, 3, 3)
    invAff = jnp.asarray(inv_affine_trans).reshape(B, V, 3, 3)
    invK = jnp.linalg.inv(K)
    ray = jnp.einsum('bvij,bvjk,kp->bvip', invK, invAff, onehm)
    deps = jnp.array([1000.0, 5000.0], jnp.float32).reshape(2, 1, 1, 1, 1)
    xg = jnp.einsum('bvji,dbvjp->dbvip', R, deps * ray[None]) + T[None]
    xcam = jnp.einsum('boij,dbcojp->dbcoip', R, xg[:, :, :, None] - T[:, None])
    xnorm = xcam / xcam[:, :, :, :, 2:3]
    M = jnp.einsum('bvij,bvjk->bvik', Aff, K)
    uv = jnp.einsum('boij,dbcojp->dbcoip', M, xnorm)
    oth = np.array([[o for o in range(V) if o != c] for c in range(V)])
    uv = uv[:, :, jnp.arange(V)[:, None], oth]
    x0, y0 = uv[0, ..., 0, :], uv[0, ..., 1, :]
    x1, y1 = uv[1, ..., 0, :], uv[1, ..., 1, :]
    kk = (y1 - y0) / (x1 - x0)
    xs = jnp.arange(w, dtype=jnp.float32)
    ysw = kk[..., None] * (xs - x0[..., None]) + y0[..., None]   # (B,V,V-1,HW,w)
    ysh = jnp.arange(h, dtype=jnp.float32)
    xsh = (ysh - y0[..., None]) / kk[..., None] + x0[..., None]  # (B,V,V-1,HW,h)

    # Reference normalizes to [-1,1] then maps back before rounding; that
    # fp round-trip shifts values by a few ulp, so replicate it exactly.
    def _round_chain(v):
        v = jnp.where(jnp.isfinite(v), v, jnp.float32(BIG))
        g = v / jnp.float32((W - 1) / 2.0) - 1.0
        return jnp.round((g + 1.0) * 0.5 * (W - 1))

    iy = np.asarray(_round_chain(ysw), np.float32)
    ix = np.asarray(_round_chain(xsh), np.float32)
    iy = iy.reshape(NPAIR, HW, W).transpose(0, 2, 1)
    ix = ix.reshape(NPAIR, HW, H).transpose(0, 2, 1)
    ctx.__exit__(None, None, None)
    return iy, ix


def _host_indices(iy, ix):
    """clamp -> fp16 index rows [12, 2(sweep), 64(t), 4096(px)]."""
    out = np.empty((NPAIR, 2, W, HW), dtype=np.float16)
    for s, arr in enumerate((iy, ix)):
        r = np.clip(arr, -1.0, 64.0)           # invalid -> never matches iota
        r = np.where(np.isfinite(r), r, 64.0)  # NaN paranoia
        out[:, s] = r.astype(np.float16)
    return out


def _host_tables(heatmaps):
    """Parity-interleaved block-diagonal fp16 two-part gather tables.

    Returns [12, 2, 32, 128, 64] fp16. Row k holds table entry k//2 of
    sweep position t = 2g + (k % 2).  Columns:
      0:16  hi, even parity   16:32 hi, odd parity    (MM1 = cols 0:32)
      32:48 lo, even parity   48:64 lo, odd parity    (MM2 = cols 32:64)
    x-sweep entry (y, t) -> hm[o, ch, y, t]; y-sweep (x, t) -> hm[o, ch, t, x].
    """
    hm = np.asarray(heatmaps, np.float32).reshape(NVIEW, C, H, W)
    hi = hm.astype(np.float16)
    lo = (hm - hi.astype(np.float32)).astype(np.float16)

    tab = np.zeros((NPAIR, 2, NTP, 128, 64), dtype=np.float16)
    for p, (c, o) in enumerate(_PAIRS):
        for part, src in ((0, hi), (1, lo)):
            base = 32 * part
            xsv = src[o].transpose(2, 1, 0)   # [t, entry(y), ch]
            ysv = src[o].transpose(1, 2, 0)   # [t'(row), entry(x), ch]
            for sweep, v in ((0, xsv), (1, ysv)):
                # even parity: t = 2g, rows 0::2, cols base+0:16
                tab[p, sweep, :, 0::2, base + 0:base + 16] = v[0::2]
                # odd parity: t = 2g+1, rows 1::2, cols base+16:32
                tab[p, sweep, :, 1::2, base + 16:base + 32] = v[1::2]
    return tab


_COMPILED = {}


def _build_program():
    import concourse.bacc as bacc
    import concourse.mybir as mybir
    import concourse.tile as tile
    from contextlib import ExitStack

    dt = mybir.dt
    ops = mybir.AluOpType
    act = mybir.ActivationFunctionType

    nc = bacc.Bacc("TRN2", target_bir_lowering=False, debug=False,
                   num_devices=NCORE)

    # idxb: [pair, sweep, chunk, 32 replicated parity rows, g_local*512+px]
    idx_d = nc.dram_tensor("idxb", [NPAIR, 2, NCH, 32, CHTP * PXS], dt.float16,
                           kind="ExternalInput")
    tab_d = nc.dram_tensor("tab", [NPAIR, 2, NTP, 128, 64], dt.float16,
                           kind="ExternalInput")
    iota_d = nc.dram_tensor("iota", [128, 1], dt.float32, kind="ExternalInput")
    niota_d = nc.dram_tensor("niota", [128, 1], dt.float32, kind="ExternalInput")
    out_d = nc.dram_tensor("out", [NPAIR, 16, PXS], dt.float32,
                           kind="ExternalOutput")

    with tile.TileContext(nc) as tc:
        with ExitStack() as ctx:
            cpool = ctx.enter_context(tc.tile_pool(name="const", bufs=1))
            tpool = ctx.enter_context(tc.tile_pool(name="tabs", bufs=4))
            rpool = ctx.enter_context(tc.tile_pool(name="repl", bufs=6))
            mpool = ctx.enter_context(tc.tile_pool(name="mask", bufs=8))
            spool = ctx.enter_context(tc.tile_pool(name="sq", bufs=3))
            apool = ctx.enter_context(tc.tile_pool(name="acc", bufs=3))
            fpool = ctx.enter_context(tc.tile_pool(name="fold", bufs=3))
            espool = ctx.enter_context(tc.tile_pool(name="res", bufs=3))
            opool = ctx.enter_context(tc.tile_pool(name="O", bufs=5, space="PSUM"))

            iota = cpool.tile([128, 1], dt.float32, tag="iota")
            niota = cpool.tile([128, 1], dt.float32, tag="niota")
            nc.sync.dma_start(iota[:], iota_d.ap())
            nc.sync.dma_start(niota[:], niota_d.ap())

            for p in range(NPAIR):
                res_ps = None
                for s in range(2):
                    tab = tpool.tile([128, NTP * 64], dt.float16, tag="tab")
                    nc.sync.dma_start(
                        tab[:].rearrange("k (g x) -> k g x", g=NTP),
                        tab_d.ap()[p, s].rearrange("g k x -> k g x"))

                    # replicate idx rows chunk-wise via DMA doubling chains
                    reps = []
                    for cch in range(NCH):
                        rep = rpool.tile([128, CHTP * PXS], dt.float16,
                                         tag="rep")
                        nc.sync.dma_start(rep[0:32, :], idx_d.ap()[p, s, cch])
                        rr = 32
                        while rr < 128:
                            nc.sync.dma_start(rep[rr:2 * rr, :], rep[0:rr, :])
                            rr *= 2
                        reps.append(rep)

                    acc = apool.tile([128, PXS], dt.float32, tag="acc")
                    for gg in range(NTP // 4):
                        ops_ps = opool.tile([128, PXS], dt.float32, tag="O")
                        for slot in range(4):
                            g = gg * 4 + slot
                            rep = reps[g // CHTP]
                            gl = g % CHTP
                            P = rep[:, gl * PXS:(gl + 1) * PXS]
                            mask = mpool.tile([128, PXS], dt.float16, tag="m")
                            if g < ACT_SHARE:
                                sq = spool.tile([128, PXS], dt.float16,
                                                tag="sq")
                                nc.scalar.activation(sq[:], P, act.Square,
                                                     bias=niota[:], scale=1.0)
                                nc.scalar.activation(mask[:], sq[:], act.Relu,
                                                     bias=1.0, scale=-1.0)
                            else:
                                nc.vector.tensor_scalar(mask[:], P, iota[:],
                                                        None, ops.is_equal)
                            tsl = tab[:, g * 64:g * 64 + 32]
                            nc.tensor.matmul(
                                ops_ps[32 * slot:32 * slot + 32, :],
                                tsl, mask[:], start=True, stop=False,
                                tile_position=(0, 32 * slot))
                            tsl2 = tab[:, g * 64 + 32:g * 64 + 64]
                            nc.tensor.matmul(
                                ops_ps[32 * slot:32 * slot + 32, :],
                                tsl2, mask[:], start=False, stop=True,
                                tile_position=(0, 32 * slot))
                        if gg == 0:
                            nc.vector.tensor_copy(acc[:], ops_ps[:])
                        else:
                            nc.vector.tensor_tensor(acc[:], acc[:], ops_ps[:],
                                                    ops.max)
                    # fold 4 col-group slots (partition blocks of 32)
                    f64 = fpool.tile([64, PXS], dt.float32, tag="f64")
                    nc.scalar.dma_start(f64[:], acc[64:128, :])
                    nc.vector.tensor_tensor(f64[:], f64[:], acc[0:64, :], ops.max)
                    f32t = fpool.tile([32, PXS], dt.float32, tag="f32")
                    nc.scalar.dma_start(f32t[:], f64[32:64, :])
                    nc.vector.tensor_tensor(f32t[:], f32t[:], f64[0:32, :], ops.max)
                    # fold channel parity blocks (16)
                    f16 = fpool.tile([16, PXS], dt.float32, tag="f16")
                    nc.scalar.dma_start(f16[:], f32t[16:32, :])
                    nc.vector.tensor_tensor(f16[:], f16[:], f32t[0:16, :], ops.max)
                    if s == 0:
                        res_ps = espool.tile([16, PXS], dt.float32, tag="res")
                        nc.vector.tensor_copy(res_ps[:], f16[:])
                    else:
                        nc.vector.tensor_tensor(res_ps[:], res_ps[:], f16[:],
                                                ops.max)
                nc.sync.dma_start(out_d.ap()[p], res_ps[:])

    nc.compile()
    return nc


def _make_in_maps(inputs):
    iy, ix = _line_coords(inputs["affine_trans"], inputs["cam_Intri"],
                          inputs["cam_R"], inputs["cam_T"],
                          inputs["inv_affine_trans"])
    idx = _host_indices(iy, ix)             # [12, 2, 64, 4096] fp16
    tab = _host_tables(inputs["heatmaps"])  # [12, 2, 32, 128, 64] fp16

    iota = (np.arange(128, dtype=np.float32) // 2).reshape(128, 1)
    niota = np.ascontiguousarray(-iota)

    in_maps = []
    for i in range(NCORE):
        sl = slice(i * PXS, (i + 1) * PXS)
        idx_i = idx[:, :, :, sl]                       # [12, 2, 64t, 512]
        # [pair, sweep, chunk, parity, g_local, px] -> replicate parity rows x16
        idxb2 = np.ascontiguousarray(
            idx_i.reshape(NPAIR, 2, NCH, CHTP, 2, PXS).transpose(0, 1, 2, 4, 3, 5)
        ).reshape(NPAIR, 2, NCH, 1, 2, CHTP * PXS)
        idxb = np.ascontiguousarray(
            np.broadcast_to(idxb2, (NPAIR, 2, NCH, 16, 2, CHTP * PXS))
        ).reshape(NPAIR, 2, NCH, 32, CHTP * PXS)
        in_maps.append({"idxb": idxb, "tab": tab,
                        "iota": iota, "niota": niota})
    return in_maps


def kernel(heatmaps, affine_trans, cam_Intri, cam_R, cam_T, inv_affine_trans):
    from concourse.bass_utils import run_bass_kernel_spmd

    heatmaps = np.asarray(heatmaps)
    in_dtype = heatmaps.dtype
    inputs = {"heatmaps": heatmaps, "affine_trans": affine_trans,
              "cam_Intri": cam_Intri, "cam_R": cam_R, "cam_T": cam_T,
              "inv_affine_trans": inv_affine_trans}

    if "prog" not in _COMPILED:
        _COMPILED["prog"] = _build_program()
    nc = _COMPILED["prog"]

    in_maps = _make_in_maps(inputs)
    res = run_bass_kernel_spmd(nc, in_maps, list(range(NCORE)))

    out = np.empty((NVIEW, NVIEW - 1, C, H, W), dtype=np.float32)
    for i in range(NCORE):
        o_i = res.results[i]["out"].reshape(NPAIR, C, ROWS, W)
        for p, (c, o) in enumerate(_PAIRS):
            slot = [v for v in range(NVIEW) if v != c].index(o)
            out[c, slot, :, i * ROWS:(i + 1) * ROWS, :] = o_i[p]
    return out.reshape(NVIEW, NVIEW - 1, C, H, W).astype(in_dtype, copy=False)



# revision 2
# speedup vs baseline: 1.0039x; 1.0039x over previous
"""Trainium2 Bass kernel for nn_CamFusionModule (epipolar max-sampling fusion).

Design (host-scheduled windowed gather):

Host (bit-exact jax-CPU camera math, as the reference):
  * per (pair, sweep, t) rounded sample indices for all 4096 pixels.
  * pixels sorted per pair by epipolar-line parameter -> 128-px blocks
    whose index values cluster into narrow y-windows.
  * work items (pair, sweep, t-pair, px-block, y-window of 16): for each,
    a one-hot fp8 mask [32=(2 parity x 16 y_off), 128 px] and an fp16
    table [32, 32=(2 parity x 16 ch)] holding the heatmap samples.
  * items are grouped by (pair, px-block), padded to 16 (= one PSUM
    bank), bin-packed across the 8 cores, and packed into DMA strips.

Device (identical SPMD program on 8 cores; only the data differs):
  stream strips -> per item one K=32 matmul (mask stationary fp8,
  table moving fp16) gathering 2x16-channel samples for 128 px into a
  PSUM bank slot; after 16 items, max-reduce the bank [128, 512] ->
  [128, 16] (DVE tensor_reduce / GPSIMD max tree, 2:1 split); batch
  results stream to DRAM.

Host combines per-group batches (max), unpermutes pixels, reassembles
[4, 3, 16, 64, 64]. Zero padding is exact: heatmaps are non-negative
and the reference floors partially-OOB lines at 0.
"""

import numpy as np
import ml_dtypes

NVIEW = 4
B, C, H, W = 1, 16, 64, 64
HW = H * W
NPAIR = 12
NCORE = 8
PXB = 128            # pixels per matmul block (M)
WIN = 16             # y-window height
BANK = 16            # items per PSUM bank / drain batch
SPF = 64             # items per strip per partition slot
SLOTS = 3            # partition slots per strip (base 0/32/64)
SITEMS = SPF * SLOTS  # 192 items per strip

_PAIRS = [(c, o) for c in range(NVIEW) for o in range(NVIEW) if o != c]
_F8 = ml_dtypes.float8_e4m3


def _line_coords(affine_trans, cam_Intri, cam_R, cam_T, inv_affine_trans):
    """Reference-exact rounded sample indices.
    Returns idx [12, 2, 64, 4096] float32 where idx[p, 0, t, px] is the
    x-sweep row index (sample hm[o, ch, idx, t]) and idx[p, 1, t, px] the
    y-sweep column index (sample hm[o, ch, t, idx]); invalid -> -1."""
    import jax
    import jax.numpy as jnp
    cpu = jax.devices("cpu")[0]
    with jax.default_device(cpu):
        V = NVIEW
        h, w = H, W
        BIG = 1.0e9
        yy, xx = jnp.meshgrid(jnp.arange(h, dtype=jnp.float32),
                              jnp.arange(w, dtype=jnp.float32), indexing='ij')
        onehm = jnp.stack([xx.reshape(-1), yy.reshape(-1),
                           jnp.ones(HW, jnp.float32)], 0)
        K = jnp.asarray(cam_Intri).reshape(B, V, 3, 3)
        R = jnp.asarray(cam_R).reshape(B, V, 3, 3)
        T = jnp.asarray(cam_T).reshape(B, V, 3, 1)
        Aff = jnp.asarray(affine_trans).reshape(B, V, 3, 3)
        invAff = jnp.asarray(inv_affine_trans).reshape(B, V, 3, 3)
        invK = jnp.linalg.inv(K)
        ray = jnp.einsum('bvij,bvjk,kp->bvip', invK, invAff, onehm)
        deps = jnp.array([1000.0, 5000.0], jnp.float32).reshape(2, 1, 1, 1, 1)
        xg = jnp.einsum('bvji,dbvjp->dbvip', R, deps * ray[None]) + T[None]
        xcam = jnp.einsum('boij,dbcojp->dbcoip', R, xg[:, :, :, None] - T[:, None])
        xnorm = xcam / xcam[:, :, :, :, 2:3]
        M = jnp.einsum('bvij,bvjk->bvik', Aff, K)
        uv = jnp.einsum('boij,dbcojp->dbcoip', M, xnorm)
        oth = np.array([[o for o in range(V) if o != c] for c in range(V)])
        uv = uv[:, :, jnp.arange(V)[:, None], oth]
        x0, y0 = uv[0, ..., 0, :], uv[0, ..., 1, :]
        x1, y1 = uv[1, ..., 0, :], uv[1, ..., 1, :]
        kk = (y1 - y0) / (x1 - x0)
        xs = jnp.arange(w, dtype=jnp.float32)
        ysw = kk[..., None] * (xs - x0[..., None]) + y0[..., None]
        ysh = jnp.arange(h, dtype=jnp.float32)
        xsh = (ysh - y0[..., None]) / kk[..., None] + x0[..., None]

        def _round_chain(v):
            v = jnp.where(jnp.isfinite(v), v, jnp.float32(BIG))
            g = v / jnp.float32((W - 1) / 2.0) - 1.0
            return jnp.round((g + 1.0) * 0.5 * (W - 1))

        iy = np.asarray(_round_chain(ysw), np.float32)  # (B,V,V-1,HW,w)
        ix = np.asarray(_round_chain(xsh), np.float32)
    iy = iy.reshape(NPAIR, HW, W).transpose(0, 2, 1)    # [12, t, px]
    ix = ix.reshape(NPAIR, HW, H).transpose(0, 2, 1)
    idx = np.stack([iy, ix], axis=1)                    # [12, 2, 64, 4096]
    raw = np.clip(idx, -3000.0, 3000.0).astype(np.float32)
    valid = (idx >= 0) & (idx <= 63)
    idx = np.where(valid, idx, -1.0).astype(np.float32)
    return idx, raw


def _schedule(idx, raw):
    """Host scheduler. Returns per-pair perms and per-core schedules.

    Each schedule is a list of batches; each batch is (group_key, items)
    with exactly BANK items (None-padded); item = (p, s, g, blk, wb)."""
    perms = np.empty((NPAIR, HW), np.int64)
    groups = {}   # (p, blk) -> list of items
    for p in range(NPAIR):
        key1 = raw[p, 0, 32]
        key2 = raw[p, 0, 48] - raw[p, 0, 16]
        perm = np.lexsort((key2, key1))
        perms[p] = perm
        for s in range(2):
            a = idx[p, s][:, perm]                      # [64, 4096]
            for blk in range(HW // PXB):
                sl = a[:, blk * PXB:(blk + 1) * PXB]
                for g in range(W // 2):
                    rows = sl[2 * g:2 * g + 2]
                    vv = rows >= 0
                    if not vv.any():
                        continue
                    vals = rows[vv]
                    lo, hi = int(vals.min()), int(vals.max())
                    for wb in range(lo, hi + 1, WIN):
                        groups.setdefault((p, blk), []).append(
                            (p, s, g, blk, wb))
    # pad each group to a multiple of BANK
    for k, items in groups.items():
        pad = (-len(items)) % BANK
        items.extend([None] * pad)
    # greedy bin-pack groups across cores
    order = sorted(groups, key=lambda k: -len(groups[k]))
    loads = [0] * NCORE
    core_groups = [[] for _ in range(NCORE)]
    for k in order:
        c = int(np.argmin(loads))
        core_groups[c].append(k)
        loads[c] += len(groups[k])

    # pack each core's banks into the drain calendar: period-7 pattern
    # [A B A B A B S]; (A,B) positions form a pair that must hold two
    # banks of the same group; S holds one bank.
    pad_bank = (None, [None] * BANK)
    core_banks = []
    for c in range(NCORE):
        rem = {}
        for k in core_groups[c]:
            items = groups[k]
            rem[k] = [items[b0:b0 + BANK]
                      for b0 in range(0, len(items), BANK)]
        seq = []
        while rem:
            t = _cal(len(seq))
            if t == 'A':
                k = max(rem, key=lambda g: len(rem[g]))
                if len(rem[k]) >= 2:
                    seq.append((k, rem[k].pop(0)))
                    seq.append((k, rem[k].pop(0)))
                else:
                    seq.append((k, rem[k].pop(0)))
                    seq.append((k, [None] * BANK))
                if not rem[k]:
                    del rem[k]
            else:  # S
                odd = [g for g in rem if len(rem[g]) % 2 == 1]
                k = min(odd, key=lambda g: len(rem[g])) if odd else \
                    max(rem, key=lambda g: len(rem[g]))
                seq.append((k, rem[k].pop(0)))
                if not rem[k]:
                    del rem[k]
        core_banks.append(seq)
    nbank = max(len(s) for s in core_banks)
    # align all cores to nbank with pad banks (calendar-safe: pads can
    # sit at any position; a pad pair or pad single drains zeros)
    while _cal(nbank) == 'B':
        nbank += 1
    for s in core_banks:
        while len(s) < nbank:
            s.append(pad_bank)
    nitem = nbank * BANK
    nstrip = -(-nitem // SITEMS)
    return perms, core_banks, nstrip, nitem


_CAL13 = ('S', 'A', 'B', 'A', 'B', 'A', 'B',
          'S', 'A', 'B', 'A', 'B', 'S')


def _cal(b):
    """Drain calendar: bank position -> 'A' (pair first), 'B' (pair
    second, drains both), 'S' (single direct drain). Singles lead each
    period so the DVE can start before the first ACT copies land."""
    return _CAL13[b % 13]


def _ndrains(nbank):
    return sum(1 for b in range(nbank) if _cal(b) in ('B', 'S'))


def _pack_core(sched, idx, perms, hmp_x, hmp_y, nstrip):
    """Build one core's strip arrays from its schedule."""
    msk = np.zeros((nstrip, 96, SPF * PXB), _F8)
    tab = np.zeros((nstrip, 96, SPF * 32), np.float16)
    yoff = np.arange(WIN, dtype=np.float32)
    it = 0
    for (gk, items) in sched:
        for item in items:
            st, loc = divmod(it, SITEMS)
            j, f = divmod(loc, SPF)
            it += 1
            if item is None:
                continue
            p, s, g, blk, wb = item
            o = _PAIRS[p][1]
            px = perms[p][blk * PXB:(blk + 1) * PXB]
            rows = idx[p, s][2 * g:2 * g + 2][:, px]      # [2, 128]
            m = (rows[:, None, :] == (wb + yoff)[None, :, None])
            msk[st, 32 * j:32 * j + 32, f * PXB:(f + 1) * PXB] = \
                m.reshape(32, PXB).astype(_F8)
            # table [32, 32]: row par*16+y, col par*16+ch (block diagonal)
            t32 = np.zeros((32, 32), np.float16)
            for par in range(2):
                t = 2 * g + par
                if s == 0:
                    blkv = hmp_x[o, :, wb:wb + WIN, t]    # [ch, y]
                else:
                    blkv = hmp_y[o, :, t, wb:wb + WIN]    # [ch, x]
                t32[par * 16:par * 16 + 16, par::2] = blkv.T
            tab[st, 32 * j:32 * j + 32, f * 32:(f + 1) * 32] = t32
    return msk, tab


_COMPILED = {}


DR_MOD = 10       # of every DR_MOD banks, DR_ASSIST drain via ACT copy
DR_ASSIST = 9


def _build_program(nstrip, nitem):
    import concourse.bacc as bacc
    import concourse.mybir as mybir
    import concourse.tile as tile
    from contextlib import ExitStack

    dt = mybir.dt
    ops = mybir.AluOpType
    nb = nitem // BANK
    nd = _ndrains(nb)

    nc = bacc.Bacc("TRN2", target_bir_lowering=False, debug=False,
                   num_devices=NCORE)
    msk_d = nc.dram_tensor("msk", [nstrip, 96, SPF * PXB], dt.float8e4,
                           kind="ExternalInput")
    tab_d = nc.dram_tensor("tab", [nstrip, 96, SPF * 32], dt.float16,
                           kind="ExternalInput")
    out_d = nc.dram_tensor("out", [128, nd * 16], dt.float16,
                           kind="ExternalOutput")

    with tile.TileContext(nc) as tc:
        with ExitStack() as ctx:
            spool = ctx.enter_context(tc.tile_pool(name="strips", bufs=4))
            ppool = ctx.enter_context(tc.tile_pool(name="banks", bufs=8,
                                                   space="PSUM"))
            apool = ctx.enter_context(tc.tile_pool(name="accs", bufs=3))
            dpool = ctx.enter_context(tc.tile_pool(name="scr", bufs=4))

            acc = None
            ps = None
            scrA = None
            dr = 0          # drain event counter
            for st in range(nstrip):
                mk = spool.tile([96, SPF * PXB], dt.float8e4, tag="mk")
                tb = spool.tile([96, SPF * 32], dt.float16, tag="tb")
                if st == 0:
                    # split first strip: unlock banks in consumption order
                    nc.sync.dma_start(mk[0:32, 0:16 * PXB],
                                      msk_d.ap()[0, 0:32, 0:16 * PXB])
                    nc.sync.dma_start(tb[0:32, 0:16 * 32],
                                      tab_d.ap()[0, 0:32, 0:16 * 32])
                    nc.sync.dma_start(mk[0:32, 16 * PXB:],
                                      msk_d.ap()[0, 0:32, 16 * PXB:])
                    nc.sync.dma_start(tb[0:32, 16 * 32:],
                                      tab_d.ap()[0, 0:32, 16 * 32:])
                    nc.sync.dma_start(mk[32:96, :], msk_d.ap()[0, 32:96, :])
                    nc.sync.dma_start(tb[32:96, :], tab_d.ap()[0, 32:96, :])
                else:
                    nc.sync.dma_start(mk[:], msk_d.ap()[st])
                    nc.sync.dma_start(tb[:], tab_d.ap()[st])
                for j in range(SLOTS):
                    for f in range(SPF):
                        k = st * SITEMS + j * SPF + f
                        if k >= nitem:
                            break
                        bs = k % BANK
                        if bs == 0:
                            ps = ppool.tile([128, 512], dt.float32, tag="bank")
                        pv = ps[:].rearrange("p (c s q) -> p c s q",
                                             c=16, s=16, q=2)
                        nc.tensor.matmul(
                            pv[:, :, bs, :],
                            mk[32 * j:32 * j + 32, PXB * f:PXB * (f + 1)],
                            tb[32 * j:32 * j + 32, 32 * f:32 * (f + 1)],
                            start=True, stop=True)
                        if bs == BANK - 1:
                            b = k // BANK
                            t = _cal(b)
                            if t == 'A':
                                scrA = dpool.tile([128, 512], dt.float16,
                                                  tag="scrA")
                                nc.scalar.copy(scrA[:], ps[:])
                                continue
                            if dr % 16 == 0:
                                acc = apool.tile([128, 256], dt.float16,
                                                 tag="acc")
                            dst = acc[:, 16 * (dr % 16):16 * (dr % 16) + 16]
                            if t == 'B':
                                scrB = dpool.tile([128, 512], dt.float16,
                                                  tag="scrB")
                                nc.scalar.copy(scrB[:], ps[:])
                                nc.vector.tensor_tensor(
                                    scrA[:], scrA[:], scrB[:], ops.max)
                                v = scrA[:].rearrange("p (c w) -> p c w",
                                                      c=16)
                                # fold the 32-wide runs down to 4 before
                                # the final reduce (fp16 2x DVE folds)
                                for w in (16, 8, 4):
                                    nc.vector.tensor_tensor(
                                        v[:, :, 0:w], v[:, :, 0:w],
                                        v[:, :, w:2 * w], ops.max)
                                nc.vector.tensor_reduce(
                                    dst, v[:, :, 0:4],
                                    mybir.AxisListType.X, ops.max)
                            else:  # 'S'
                                v = ps[:].rearrange("p (c w) -> p c w", c=16)
                                nc.vector.tensor_reduce(
                                    dst, v, mybir.AxisListType.X, ops.max)
                            dr += 1
                            if dr % 16 == 0 or dr == nd:
                                d0 = 16 * ((dr - 1) // 16)
                                nc.gpsimd.dma_start(
                                    out_d.ap()[:, 16 * d0:16 * dr],
                                    acc[:, 0:16 * (dr - d0)])
    nc.compile()
    return nc


def kernel(heatmaps, affine_trans, cam_Intri, cam_R, cam_T, inv_affine_trans):
    from concourse.bass_utils import run_bass_kernel_spmd

    heatmaps = np.asarray(heatmaps)
    in_dtype = heatmaps.dtype

    idx, raw = _line_coords(affine_trans, cam_Intri, cam_R, cam_T,
                            inv_affine_trans)
    perms, scheds, nstrip, nitem = _schedule(idx, raw)

    hm16 = np.asarray(heatmaps, np.float32).reshape(NVIEW, C, H, W)
    hm16 = hm16.astype(np.float16)
    # zero-pad so y-windows may overhang past 63
    hmp_x = np.zeros((NVIEW, C, H + WIN, W), np.float16)
    hmp_x[:, :, :H, :] = hm16
    hmp_y = np.zeros((NVIEW, C, H, W + WIN), np.float16)
    hmp_y[:, :, :, :W] = hm16

    in_maps = []
    for ci in range(NCORE):
        msk, tab = _pack_core(scheds[ci], idx, perms, hmp_x, hmp_y, nstrip)
        in_maps.append({"msk": msk, "tab": tab})

    key = (nstrip, nitem)
    if key not in _COMPILED:
        _COMPILED[key] = _build_program(nstrip, nitem)
    nc = _COMPILED[key]

    res = run_bass_kernel_spmd(nc, in_maps, list(range(NCORE)))

    out_full = np.zeros((NPAIR, C, HW), np.float32)
    for ci in range(NCORE):
        ob = res.results[ci]["out"].astype(np.float32)
        agg = {}
        dr = 0
        for b, (gk, items) in enumerate(scheds[ci]):
            if _cal(b) == 'A':
                continue
            ev = ob[:, 16 * dr:16 * dr + 16]
            dr += 1
            if gk is None:
                continue
            if gk in agg:
                np.maximum(agg[gk], ev, out=agg[gk])
            else:
                agg[gk] = ev.copy()
        for (p, blk), cur in agg.items():
            px = perms[p][blk * PXB:(blk + 1) * PXB]
            out_full[p][:, px] = cur.T

    out = np.zeros((NVIEW, NVIEW - 1, C, H, W), np.float32)
    for p, (c, o) in enumerate(_PAIRS):
        slot = [v for v in range(NVIEW) if v != c].index(o)
        out[c, slot] = out_full[p].reshape(C, H, W)
    return out.astype(in_dtype, copy=False)


# revision 3
# speedup vs baseline: 1.0171x; 1.0131x over previous
"""Trainium2 Bass kernel for nn_CamFusionModule (epipolar max-sampling fusion).

Design (host-scheduled windowed gather):

Host (bit-exact jax-CPU camera math, as the reference):
  * per (pair, sweep, t) rounded sample indices for all 4096 pixels.
  * pixels sorted per pair by epipolar-line parameter -> 128-px blocks
    whose index values cluster into narrow y-windows.
  * work items (pair, sweep, t-pair, px-block, y-window of 16): for each,
    a one-hot fp8 mask [32=(2 parity x 16 y_off), 128 px] and an fp16
    table [32, 32=(2 parity x 16 ch)] holding the heatmap samples.
  * items are grouped by (pair, px-block), padded to 16 (= one PSUM
    bank), bin-packed across the 8 cores, and packed into DMA strips.

Device (identical SPMD program on 8 cores; only the data differs):
  stream strips -> per item one K=32 matmul (mask stationary fp8,
  table moving fp16) gathering 2x16-channel samples for 128 px into a
  PSUM bank slot; after 16 items, max-reduce the bank [128, 512] ->
  [128, 16] (DVE tensor_reduce / GPSIMD max tree, 2:1 split); batch
  results stream to DRAM.

Host combines per-group batches (max), unpermutes pixels, reassembles
[4, 3, 16, 64, 64]. Zero padding is exact: heatmaps are non-negative
and the reference floors partially-OOB lines at 0.
"""

import numpy as np
import ml_dtypes

NVIEW = 4
B, C, H, W = 1, 16, 64, 64
HW = H * W
NPAIR = 12
NCORE = 8
PXB = 128            # pixels per matmul block (M)
WIN = 16             # y-window height
BANK = 16            # items per PSUM bank / drain batch
SPF = 64             # items per strip per partition slot
SLOTS = 3            # partition slots per strip (base 0/32/64)
SITEMS = SPF * SLOTS  # 192 items per strip

_PAIRS = [(c, o) for c in range(NVIEW) for o in range(NVIEW) if o != c]
_F8 = ml_dtypes.float8_e4m3


def _line_coords(affine_trans, cam_Intri, cam_R, cam_T, inv_affine_trans):
    """Reference-exact rounded sample indices.
    Returns idx [12, 2, 64, 4096] float32 where idx[p, 0, t, px] is the
    x-sweep row index (sample hm[o, ch, idx, t]) and idx[p, 1, t, px] the
    y-sweep column index (sample hm[o, ch, t, idx]); invalid -> -1."""
    import jax
    import jax.numpy as jnp
    cpu = jax.devices("cpu")[0]
    with jax.default_device(cpu):
        V = NVIEW
        h, w = H, W
        BIG = 1.0e9
        yy, xx = jnp.meshgrid(jnp.arange(h, dtype=jnp.float32),
                              jnp.arange(w, dtype=jnp.float32), indexing='ij')
        onehm = jnp.stack([xx.reshape(-1), yy.reshape(-1),
                           jnp.ones(HW, jnp.float32)], 0)
        K = jnp.asarray(cam_Intri).reshape(B, V, 3, 3)
        R = jnp.asarray(cam_R).reshape(B, V, 3, 3)
        T = jnp.asarray(cam_T).reshape(B, V, 3, 1)
        Aff = jnp.asarray(affine_trans).reshape(B, V, 3, 3)
        invAff = jnp.asarray(inv_affine_trans).reshape(B, V, 3, 3)
        invK = jnp.linalg.inv(K)
        ray = jnp.einsum('bvij,bvjk,kp->bvip', invK, invAff, onehm)
        deps = jnp.array([1000.0, 5000.0], jnp.float32).reshape(2, 1, 1, 1, 1)
        xg = jnp.einsum('bvji,dbvjp->dbvip', R, deps * ray[None]) + T[None]
        xcam = jnp.einsum('boij,dbcojp->dbcoip', R, xg[:, :, :, None] - T[:, None])
        xnorm = xcam / xcam[:, :, :, :, 2:3]
        M = jnp.einsum('bvij,bvjk->bvik', Aff, K)
        uv = jnp.einsum('boij,dbcojp->dbcoip', M, xnorm)
        oth = np.array([[o for o in range(V) if o != c] for c in range(V)])
        uv = uv[:, :, jnp.arange(V)[:, None], oth]
        x0, y0 = uv[0, ..., 0, :], uv[0, ..., 1, :]
        x1, y1 = uv[1, ..., 0, :], uv[1, ..., 1, :]
        kk = (y1 - y0) / (x1 - x0)
        xs = jnp.arange(w, dtype=jnp.float32)
        ysw = kk[..., None] * (xs - x0[..., None]) + y0[..., None]
        ysh = jnp.arange(h, dtype=jnp.float32)
        xsh = (ysh - y0[..., None]) / kk[..., None] + x0[..., None]

        def _round_chain(v):
            v = jnp.where(jnp.isfinite(v), v, jnp.float32(BIG))
            g = v / jnp.float32((W - 1) / 2.0) - 1.0
            return jnp.round((g + 1.0) * 0.5 * (W - 1))

        iy = np.asarray(_round_chain(ysw), np.float32)  # (B,V,V-1,HW,w)
        ix = np.asarray(_round_chain(xsh), np.float32)
    iy = iy.reshape(NPAIR, HW, W).transpose(0, 2, 1)    # [12, t, px]
    ix = ix.reshape(NPAIR, HW, H).transpose(0, 2, 1)
    idx = np.stack([iy, ix], axis=1)                    # [12, 2, 64, 4096]
    raw = np.clip(idx, -3000.0, 3000.0).astype(np.float32)
    valid = (idx >= 0) & (idx <= 63)
    idx = np.where(valid, idx, -1.0).astype(np.float32)
    return idx, raw


def _schedule(idx, raw):
    """Host scheduler. Returns per-pair perms and per-core schedules.

    Each schedule is a list of batches; each batch is (group_key, items)
    with exactly BANK items (None-padded); item = (p, s, g, blk, wb)."""
    perms = np.empty((NPAIR, HW), np.int64)
    groups = {}   # (p, blk) -> list of items
    for p in range(NPAIR):
        key1 = raw[p, 0, 32]
        key2 = raw[p, 0, 48] - raw[p, 0, 16]
        perm = np.lexsort((key2, key1))
        perms[p] = perm
        for s in range(2):
            a = idx[p, s][:, perm]                      # [64, 4096]
            for blk in range(HW // PXB):
                sl = a[:, blk * PXB:(blk + 1) * PXB]
                for g in range(W // 2):
                    rows = sl[2 * g:2 * g + 2]
                    vv = rows >= 0
                    if not vv.any():
                        continue
                    vals = rows[vv]
                    lo, hi = int(vals.min()), int(vals.max())
                    for wb in range(lo, hi + 1, WIN):
                        groups.setdefault((p, blk), []).append(
                            (p, s, g, blk, wb))
    # pad each group to a multiple of BANK
    for k, items in groups.items():
        pad = (-len(items)) % BANK
        items.extend([None] * pad)
    # greedy bin-pack groups across cores
    order = sorted(groups, key=lambda k: -len(groups[k]))
    loads = [0] * NCORE
    core_groups = [[] for _ in range(NCORE)]
    for k in order:
        c = int(np.argmin(loads))
        core_groups[c].append(k)
        loads[c] += len(groups[k])

    # pack each core's banks into the drain calendar: period-7 pattern
    # [A B A B A B S]; (A,B) positions form a pair that must hold two
    # banks of the same group; S holds one bank.
    pad_bank = (None, [None] * BANK)
    core_banks = []
    for c in range(NCORE):
        rem = {}
        for k in core_groups[c]:
            items = groups[k]
            rem[k] = [items[b0:b0 + BANK]
                      for b0 in range(0, len(items), BANK)]
        seq = []
        while rem:
            t = _cal(len(seq))
            if t == 'A':
                k = max(rem, key=lambda g: len(rem[g]))
                if len(rem[k]) >= 2:
                    seq.append((k, rem[k].pop(0)))
                    seq.append((k, rem[k].pop(0)))
                else:
                    seq.append((k, rem[k].pop(0)))
                    seq.append((k, [None] * BANK))
                if not rem[k]:
                    del rem[k]
            else:  # S
                odd = [g for g in rem if len(rem[g]) % 2 == 1]
                k = min(odd, key=lambda g: len(rem[g])) if odd else \
                    max(rem, key=lambda g: len(rem[g]))
                seq.append((k, rem[k].pop(0)))
                if not rem[k]:
                    del rem[k]
        core_banks.append(seq)
    nbank = max(len(s) for s in core_banks)
    # align all cores to nbank with pad banks (calendar-safe: pads can
    # sit at any position; a pad pair or pad single drains zeros)
    while _cal(nbank) == 'B':
        nbank += 1
    for s in core_banks:
        while len(s) < nbank:
            s.append(pad_bank)
    nitem = nbank * BANK
    nstrip = -(-nitem // SITEMS)
    return perms, core_banks, nstrip, nitem


_CAL13 = ('S', 'A', 'B', 'A', 'B', 'A', 'B',
          'S', 'A', 'B', 'A', 'B', 'S')


def _cal(b):
    """Drain calendar: bank position -> 'A' (pair first), 'B' (pair
    second, drains both), 'S' (single direct drain). Singles lead each
    period so the DVE can start before the first ACT copies land."""
    return _CAL13[b % 13]


def _ndrains(nbank):
    return sum(1 for b in range(nbank) if _cal(b) in ('B', 'S'))


def _pack_core(sched, idx, perms, hmp_x, hmp_y, nstrip):
    """Build one core's strip arrays from its schedule."""
    msk = np.zeros((nstrip, 96, SPF * PXB), _F8)
    tab = np.zeros((nstrip, 96, SPF * 32), np.float16)
    yoff = np.arange(WIN, dtype=np.float32)
    it = 0
    for (gk, items) in sched:
        for item in items:
            st, loc = divmod(it, SITEMS)
            j, f = divmod(loc, SPF)
            it += 1
            if item is None:
                continue
            p, s, g, blk, wb = item
            o = _PAIRS[p][1]
            px = perms[p][blk * PXB:(blk + 1) * PXB]
            rows = idx[p, s][2 * g:2 * g + 2][:, px]      # [2, 128]
            m = (rows[:, None, :] == (wb + yoff)[None, :, None])
            msk[st, 32 * j:32 * j + 32, f * PXB:(f + 1) * PXB] = \
                m.reshape(32, PXB).astype(_F8)
            # table [32, 32]: row par*16+y, col par*16+ch (block diagonal)
            t32 = np.zeros((32, 32), np.float16)
            for par in range(2):
                t = 2 * g + par
                if s == 0:
                    blkv = hmp_x[o, :, wb:wb + WIN, t]    # [ch, y]
                else:
                    blkv = hmp_y[o, :, t, wb:wb + WIN]    # [ch, x]
                t32[par * 16:par * 16 + 16, par::2] = blkv.T
            tab[st, 32 * j:32 * j + 32, f * 32:(f + 1) * 32] = t32
    return msk, tab


_COMPILED = {}


DR_MOD = 10       # of every DR_MOD banks, DR_ASSIST drain via ACT copy
DR_ASSIST = 9


def _build_program(nstrip, nitem):
    import concourse.bacc as bacc
    import concourse.mybir as mybir
    import concourse.tile as tile
    from contextlib import ExitStack

    dt = mybir.dt
    ops = mybir.AluOpType
    nb = nitem // BANK
    nd = _ndrains(nb)

    nc = bacc.Bacc("TRN2", target_bir_lowering=False, debug=False,
                   num_devices=NCORE)
    msk_d = nc.dram_tensor("msk", [nstrip, 96, SPF * PXB], dt.float8e4,
                           kind="ExternalInput")
    tab_d = nc.dram_tensor("tab", [nstrip, 96, SPF * 32], dt.float16,
                           kind="ExternalInput")
    out_d = nc.dram_tensor("out", [128, nd * 16], dt.float16,
                           kind="ExternalOutput")

    with tile.TileContext(nc) as tc:
        with ExitStack() as ctx:
            spool = ctx.enter_context(tc.tile_pool(name="strips", bufs=5))
            ppool = ctx.enter_context(tc.tile_pool(name="banks", bufs=8,
                                                   space="PSUM"))
            apool = ctx.enter_context(tc.tile_pool(name="accs", bufs=4))
            dpool = ctx.enter_context(tc.tile_pool(name="scr", bufs=6))

            acc = None
            ps = None
            scrA = None
            dr = 0          # drain event counter
            for st in range(nstrip):
                mk = spool.tile([96, SPF * PXB], dt.float8e4, tag="mk")
                tb = spool.tile([96, SPF * 32], dt.float16, tag="tb")
                if st == 0:
                    # split first strip: unlock banks in consumption order
                    nc.sync.dma_start(mk[0:32, 0:16 * PXB],
                                      msk_d.ap()[0, 0:32, 0:16 * PXB])
                    nc.sync.dma_start(tb[0:32, 0:16 * 32],
                                      tab_d.ap()[0, 0:32, 0:16 * 32])
                    nc.sync.dma_start(mk[0:32, 16 * PXB:],
                                      msk_d.ap()[0, 0:32, 16 * PXB:])
                    nc.sync.dma_start(tb[0:32, 16 * 32:],
                                      tab_d.ap()[0, 0:32, 16 * 32:])
                    nc.sync.dma_start(mk[32:96, :], msk_d.ap()[0, 32:96, :])
                    nc.sync.dma_start(tb[32:96, :], tab_d.ap()[0, 32:96, :])
                else:
                    nc.sync.dma_start(mk[:], msk_d.ap()[st])
                    nc.sync.dma_start(tb[:], tab_d.ap()[st])
                for j in range(SLOTS):
                    for f in range(SPF):
                        k = st * SITEMS + j * SPF + f
                        if k >= nitem:
                            break
                        bs = k % BANK
                        if bs == 0:
                            ps = ppool.tile([128, 512], dt.float32, tag="bank")
                        pv = ps[:].rearrange("p (c s q) -> p c s q",
                                             c=16, s=16, q=2)
                        nc.tensor.matmul(
                            pv[:, :, bs, :],
                            mk[32 * j:32 * j + 32, PXB * f:PXB * (f + 1)],
                            tb[32 * j:32 * j + 32, 32 * f:32 * (f + 1)],
                            start=True, stop=True)
                        if bs == BANK - 1:
                            b = k // BANK
                            t = _cal(b)
                            if t == 'A':
                                scrA = dpool.tile([128, 512], dt.float16,
                                                  tag="scrA")
                                nc.scalar.copy(scrA[:], ps[:])
                                continue
                            if dr % 16 == 0:
                                acc = apool.tile([128, 256], dt.float16,
                                                 tag="acc")
                            dst = acc[:, 16 * (dr % 16):16 * (dr % 16) + 16]
                            if t == 'B':
                                scrB = dpool.tile([128, 512], dt.float16,
                                                  tag="scrB")
                                nc.scalar.copy(scrB[:], ps[:])
                                nc.vector.tensor_tensor(
                                    scrA[:], scrA[:], scrB[:], ops.max)
                                v = scrA[:].rearrange("p (c w) -> p c w",
                                                      c=16)
                                # fold the 32-wide runs down to 4 before
                                # the final reduce (fp16 2x DVE folds)
                                for w in (16, 8, 4):
                                    nc.vector.tensor_tensor(
                                        v[:, :, 0:w], v[:, :, 0:w],
                                        v[:, :, w:2 * w], ops.max)
                                nc.vector.tensor_reduce(
                                    dst, v[:, :, 0:4],
                                    mybir.AxisListType.X, ops.max)
                            else:  # 'S'
                                v = ps[:].rearrange("p (c w) -> p c w", c=16)
                                nc.vector.tensor_reduce(
                                    dst, v, mybir.AxisListType.X, ops.max)
                            dr += 1
                            if dr % 16 == 0 or dr == nd:
                                d0 = 16 * ((dr - 1) // 16)
                                nc.gpsimd.dma_start(
                                    out_d.ap()[:, 16 * d0:16 * dr],
                                    acc[:, 0:16 * (dr - d0)])
    nc.compile()
    return nc


def kernel(heatmaps, affine_trans, cam_Intri, cam_R, cam_T, inv_affine_trans):
    from concourse.bass_utils import run_bass_kernel_spmd

    heatmaps = np.asarray(heatmaps)
    in_dtype = heatmaps.dtype

    idx, raw = _line_coords(affine_trans, cam_Intri, cam_R, cam_T,
                            inv_affine_trans)
    perms, scheds, nstrip, nitem = _schedule(idx, raw)

    hm16 = np.asarray(heatmaps, np.float32).reshape(NVIEW, C, H, W)
    hm16 = hm16.astype(np.float16)
    # zero-pad so y-windows may overhang past 63
    hmp_x = np.zeros((NVIEW, C, H + WIN, W), np.float16)
    hmp_x[:, :, :H, :] = hm16
    hmp_y = np.zeros((NVIEW, C, H, W + WIN), np.float16)
    hmp_y[:, :, :, :W] = hm16

    in_maps = []
    for ci in range(NCORE):
        msk, tab = _pack_core(scheds[ci], idx, perms, hmp_x, hmp_y, nstrip)
        in_maps.append({"msk": msk, "tab": tab})

    key = (nstrip, nitem)
    if key not in _COMPILED:
        _COMPILED[key] = _build_program(nstrip, nitem)
    nc = _COMPILED[key]

    res = run_bass_kernel_spmd(nc, in_maps, list(range(NCORE)))

    out_full = np.zeros((NPAIR, C, HW), np.float32)
    for ci in range(NCORE):
        ob = res.results[ci]["out"].astype(np.float32)
        agg = {}
        dr = 0
        for b, (gk, items) in enumerate(scheds[ci]):
            if _cal(b) == 'A':
                continue
            ev = ob[:, 16 * dr:16 * dr + 16]
            dr += 1
            if gk is None:
                continue
            if gk in agg:
                np.maximum(agg[gk], ev, out=agg[gk])
            else:
                agg[gk] = ev.copy()
        for (p, blk), cur in agg.items():
            px = perms[p][blk * PXB:(blk + 1) * PXB]
            out_full[p][:, px] = cur.T

    out = np.zeros((NVIEW, NVIEW - 1, C, H, W), np.float32)
    for p, (c, o) in enumerate(_PAIRS):
        slot = [v for v in range(NVIEW) if v != c].index(o)
        out[c, slot] = out_full[p].reshape(C, H, W)
    return out.astype(in_dtype, copy=False)
